# revision 12
# baseline (speedup 1.0000x reference)
"""Trainium2 Bass kernel for a dense transformer block (pre-LN, MHA + MLP).

Sharding: 8 cores; core c handles batch b = c // 4, query block qb = c % 4
(512 tokens). Each core recomputes K/V for its batch's full 2048-token
sequence (zero cross-core communication), then runs attention for its
512 query tokens and the MLP on them.

All activations are kept feature-major ([feature, token]) on device, and the
host pre-transposes x and all weights, so every matmul is layout-natural
(contraction dim on partitions) with no device transposes. Softmax runs
without max-subtraction (scores are small: |q.k|/8 <~ 5), with denominators
produced by a ones-row appended to V inside the AV matmul. Matmuls use
float32r (full PE rate at free-dim >= 256, ~1e-4 relative error).
"""
import numpy as np

import concourse.bass as bass
import concourse.mybir as mybir
import concourse.tile as tile
from concourse import bacc
from concourse.bass_utils import run_bass_kernel_spmd

P = 128
C = 1024
NCT = C // P          # 8 feature tiles
TKV = 2048            # kv tokens per core (sequence length)
TQ = 512              # query tokens per core
HID = 4096
NHT = HID // P        # 32 hidden tiles
H = 16
HD = 64
NHP = H // 2          # 8 head pairs
CHUNK = 512           # kv tokens processed per pipeline chunk
NCHUNK = TKV // CHUNK # 4
NJCL = CHUNK // P     # 4 j-subchunks of 128 per chunk
EPS = 1e-5
SCALE = HD ** -0.5

f32 = mybir.dt.float32
f32r = mybir.dt.float32r
Act = mybir.ActivationFunctionType


def _r(ap):
    return ap.bitcast(f32r)


def _emit_ln(nc, ones_stat, eps_t, ps_pool, sb_pool, x_of_ct, out, g_t, b_t,
             F):
    """LayerNorm over the feature (partition) dim for one <=512-token chunk.

    x_of_ct(ct) -> [128, F] input AP; out: [128, NCT, F] tile; g_t/b_t:
    [128, NCT] per-feature scale/bias tiles.
    """
    ps_stat = ps_pool.tile([1, 2 * F], f32, tag="ln_stat", bufs=1)
    for ct in range(NCT):
        x_ct = x_of_ct(ct)
        sq = sb_pool.tile([P, F], f32, tag="ln_sq", bufs=3)
        nc.vector.tensor_mul(sq[:], x_ct, x_ct)
        nc.tensor.matmul(ps_stat[:, 0:F], _r(ones_stat[:]), _r(x_ct),
                         start=(ct == 0), stop=(ct == NCT - 1))
        nc.tensor.matmul(ps_stat[:, F:2 * F], _r(ones_stat[:]), _r(sq[:]),
                         start=(ct == 0), stop=(ct == NCT - 1))
    mu = sb_pool.tile([1, F], f32, tag="ln_mu", bufs=1)
    var = sb_pool.tile([1, F], f32, tag="ln_var", bufs=1)
    nc.vector.tensor_scalar_mul(mu[:], ps_stat[:, 0:F], 1.0 / C)
    nc.vector.tensor_scalar_mul(var[:], ps_stat[:, F:2 * F], 1.0 / C)
    mu2 = sb_pool.tile([1, F], f32, tag="ln_mu2", bufs=1)
    nc.vector.tensor_mul(mu2[:], mu[:], mu[:])
    nc.vector.tensor_sub(var[:], var[:], mu2[:])
    # rstd = 1 / sqrt(var + eps)
    nc.scalar.activation(var[:], var[:], Act.Sqrt, bias=eps_t[0:1, :])
    nc.vector.reciprocal(var[:], var[:])
    mu_b = sb_pool.tile([P, F], f32, tag="ln_mub")
    rstd_b = sb_pool.tile([P, F], f32, tag="ln_rstdb")
    nc.gpsimd.partition_broadcast(mu_b[:], mu[:])
    nc.gpsimd.partition_broadcast(rstd_b[:], var[:])
    for ct in range(NCT):
        o = out[:, ct, :]
        nc.vector.tensor_sub(o, x_of_ct(ct), mu_b[:])
        nc.vector.tensor_mul(o, o, rstd_b[:])
        nc.vector.tensor_scalar(o, o, g_t[:, ct:ct + 1], b_t[:, ct:ct + 1],
                                op0=mybir.AluOpType.mult,
                                op1=mybir.AluOpType.add)


def build_program(sim_standin=False):
    # CoreSim lacks Gelu; Tanh has identical ACT cost, so the sim variant
    # swaps it in for modeled-time runs (numerics then checked vs a matching
    # numpy reference).
    gelu_fn = Act.Tanh if sim_standin else Act.Gelu
    nc = bacc.Bacc()

    # DRAM I/O (per core). All feature-major / pre-transposed by the host.
    xkvT = nc.dram_tensor("xkvT", [C, TKV], f32, kind="ExternalInput")
    xqT = nc.dram_tensor("xqT", [C, TQ], f32, kind="ExternalInput")
    wqT = nc.dram_tensor("wqT", [C, C], f32, kind="ExternalInput")
    wkT = nc.dram_tensor("wkT", [C, C], f32, kind="ExternalInput")
    wvT = nc.dram_tensor("wvT", [C, C], f32, kind="ExternalInput")
    wpT = nc.dram_tensor("wpT", [C, C], f32, kind="ExternalInput")
    w1T = nc.dram_tensor("w1T", [C, HID], f32, kind="ExternalInput")
    w2T = nc.dram_tensor("w2T", [HID, C], f32, kind="ExternalInput")
    bp = nc.dram_tensor("bp", [C], f32, kind="ExternalInput")
    b1 = nc.dram_tensor("b1", [HID], f32, kind="ExternalInput")
    b2 = nc.dram_tensor("b2", [C], f32, kind="ExternalInput")
    ln1g = nc.dram_tensor("ln1g", [C], f32, kind="ExternalInput")
    ln1b = nc.dram_tensor("ln1b", [C], f32, kind="ExternalInput")
    ln2g = nc.dram_tensor("ln2g", [C], f32, kind="ExternalInput")
    ln2b = nc.dram_tensor("ln2b", [C], f32, kind="ExternalInput")
    outT = nc.dram_tensor("outT", [C, TQ], f32, kind="ExternalOutput")

    with tile.TileContext(nc) as tc:
      with (
          tc.tile_pool(name="const", bufs=1) as const,
          tc.tile_pool(name="px2", bufs=1) as px2,
      ):
        ones_stat = const.tile([P, 1], f32)
        nc.vector.memset(ones_stat[:], 1.0)
        eps_t = const.tile([P, 1], f32)
        nc.vector.memset(eps_t[:], EPS)

        def vec_param(t, n, name):
            v = const.tile([P, n // P], f32, name=name)
            nc.sync.dma_start(v[:], t[:].rearrange("(ct p) -> p ct", p=P))
            return v

        g1_t = vec_param(ln1g, C, "g1_t")
        bb1_t = vec_param(ln1b, C, "bb1_t")
        g2_t = vec_param(ln2g, C, "g2_t")
        bb2_t = vec_param(ln2b, C, "bb2_t")
        bp_t = vec_param(bp, C, "bp_t")
        b1_t = vec_param(b1, HID, "b1_t")
        b2_t = vec_param(b2, C, "b2_t")

        x2T = px2.tile([P, NCT, TQ], f32)  # attn residual stream (phases C-D)

        with tc.tile_pool(name="persist", bufs=1) as persist:
            xq_t = persist.tile([P, NCT, TQ], f32)   # residual + LN1 input
            qT = persist.tile([P, NHP, TQ], f32)     # Q, feature-major
            attnT = persist.tile([P, NHP, TQ], f32)  # AV accum (unnorm.)
            # softmax denominators: head h -> partition 32*(h%4), slot h//4
            den = persist.tile([P, 4, TQ], f32)

            for ct in range(NCT):
                nc.sync.dma_start(xq_t[:, ct, :], xqT[ct * P:(ct + 1) * P, :])

            # ---- Phase A: LN1(xq) and Q projection ----
            with (
                tc.tile_pool(name="pa_sb", bufs=2) as pa_sb,
                tc.tile_pool(name="pa_w", bufs=1) as pa_w,
                tc.tile_pool(name="pa_ln", bufs=1) as pa_ln,
                tc.tile_pool(name="pa_ps", bufs=2, space="PSUM") as pa_ps,
            ):
                ln1q = pa_ln.tile([P, NCT, TQ], f32)
                _emit_ln(nc, ones_stat, eps_t, pa_ps, pa_sb,
                         lambda ct: xq_t[:, ct, :], ln1q, g1_t, bb1_t, TQ)
                wq_t = pa_w.tile([P, NCT, C], f32)
                for ct in range(NCT):
                    nc.sync.dma_start(wq_t[:, ct, :],
                                      wqT[ct * P:(ct + 1) * P, :])
                for hp in range(NHP):
                    ps = pa_ps.tile([P, TQ], f32, tag="q_ps")
                    for ct in range(NCT):
                        nc.tensor.matmul(
                            ps[:], _r(wq_t[:, ct, hp * P:(hp + 1) * P]),
                            _r(ln1q[:, ct, :]),
                            start=(ct == 0), stop=(ct == NCT - 1))
                    nc.vector.tensor_copy(qT[:, hp, :], ps[:])

            # ---- Phase B: per 512-token chunk: LN1, K, V, attention ----
            with (
                tc.tile_pool(name="pb_x", bufs=2) as pb_x,
                tc.tile_pool(name="pb_ln", bufs=1) as pb_ln,
                tc.tile_pool(name="pb_w", bufs=2) as pb_w,
                tc.tile_pool(name="pb_kv", bufs=1) as pb_kv,
                tc.tile_pool(name="pb_e", bufs=2) as pb_e,
                tc.tile_pool(name="pb_sb", bufs=2) as pb_sb,
                tc.tile_pool(name="pb_ps", bufs=1, space="PSUM") as pb_ps,
                tc.tile_pool(name="pb_psav", bufs=1, space="PSUM") as pb_psav,
            ):
                for ch in range(NCHUNK):
                    j0 = ch * CHUNK
                    xkv_t = pb_x.tile([P, NCT, CHUNK], f32, tag="xkv")
                    for ct in range(NCT):
                        nc.sync.dma_start(
                            xkv_t[:, ct, :],
                            xkvT[ct * P:(ct + 1) * P, j0:j0 + CHUNK])
                    lnkv = pb_ln.tile([P, NCT, CHUNK], f32, tag="lnkv")
                    _emit_ln(nc, ones_stat, eps_t, pb_ps, pb_sb,
                             lambda ct: xkv_t[:, ct, :], lnkv, g1_t, bb1_t,
                             CHUNK)

                    # K projection for this chunk: kT_c [128(2*64 d), hp, j]
                    kT_c = pb_kv.tile([P, NHP, CHUNK], f32, tag="kT")
                    for hp in range(NHP):
                        wk_t = pb_w.tile([P, NCT, P], f32, tag="wk")
                        for ct in range(NCT):
                            nc.sync.dma_start(
                                wk_t[:, ct, :],
                                wkT[ct * P:(ct + 1) * P,
                                    hp * P:(hp + 1) * P])
                        ps = pb_ps.tile([P, CHUNK], f32, tag="kv_ps", bufs=2)
                        for ct in range(NCT):
                            nc.tensor.matmul(
                                ps[:], _r(wk_t[:, ct, :]),
                                _r(lnkv[:, ct, :]),
                                start=(ct == 0), stop=(ct == NCT - 1))
                        nc.vector.tensor_copy(kT_c[:, hp, :], ps[:])

                    # V projection, token-major with ones column: [j, h, 65]
                    v_c = pb_kv.tile([P, NJCL, H, HD + 1], f32, tag="v")
                    for jl in range(NJCL):
                        nc.vector.memset(v_c[:, jl, :, HD:HD + 1], 1.0)
                    for half in range(2):
                        wv_t = pb_w.tile([P, NCT, 512], f32, tag="wv", bufs=1)
                        for ct in range(NCT):
                            nc.sync.dma_start(
                                wv_t[:, ct, :],
                                wvT[ct * P:(ct + 1) * P,
                                    half * 512:(half + 1) * 512])
                        for jl in range(NJCL):
                            ps = pb_ps.tile([P, CHUNK], f32, tag="kv_ps",
                                            bufs=2)
                            for ct in range(NCT):
                                nc.tensor.matmul(
                                    ps[:],
                                    _r(lnkv[:, ct, jl * P:(jl + 1) * P]),
                                    _r(wv_t[:, ct, :]),
                                    start=(ct == 0), stop=(ct == NCT - 1))
                            nc.vector.tensor_copy(
                                v_c[:, jl, half * 8:(half + 1) * 8, 0:HD],
                                ps[:].rearrange("p (h d) -> p h d", d=HD))

                    # Attention: scores -> exp -> AV accumulation
                    for hp in range(NHP):
                        ps_av0 = pb_psav.tile([HD + 1, TQ], f32, tag="av0")
                        ps_av1 = pb_psav.tile([HD + 1, TQ], f32, tag="av1")
                        for jl in range(NJCL):
                            ps_sc = pb_ps.tile([P, 2 * TQ], f32, tag="sc_ps",
                                               bufs=1)
                            nc.tensor.matmul(
                                ps_sc[:, 0:TQ],
                                _r(kT_c[0:HD, hp, jl * P:(jl + 1) * P]),
                                _r(qT[0:HD, hp, :]), start=True, stop=True)
                            nc.tensor.matmul(
                                ps_sc[:, TQ:2 * TQ],
                                _r(kT_c[HD:P, hp, jl * P:(jl + 1) * P]),
                                _r(qT[HD:P, hp, :]), start=True, stop=True)
                            e_sb = pb_e.tile([P, 2 * TQ], f32, tag="e")
                            nc.scalar.activation(e_sb[:], ps_sc[:], Act.Exp,
                                                 scale=SCALE)
                            nc.tensor.matmul(
                                ps_av0[:], _r(v_c[:, jl, 2 * hp, :]),
                                _r(e_sb[:, 0:TQ]),
                                start=(jl == 0), stop=(jl == NJCL - 1))
                            nc.tensor.matmul(
                                ps_av1[:], _r(v_c[:, jl, 2 * hp + 1, :]),
                                _r(e_sb[:, TQ:2 * TQ]),
                                start=(jl == 0), stop=(jl == NJCL - 1))
                        # accumulate into attnT / den across chunks
                        for i, ps_av in ((0, ps_av0), (1, ps_av1)):
                            h = 2 * hp + i
                            a_dst = attnT[i * HD:(i + 1) * HD, hp, :]
                            dp = 32 * (h % 4)
                            d_dst = den[dp:dp + 1, h // 4, :]
                            if ch == 0:
                                nc.vector.tensor_copy(a_dst, ps_av[0:HD, :])
                                nc.vector.tensor_copy(d_dst,
                                                      ps_av[HD:HD + 1, :])
                            else:
                                nc.vector.tensor_add(a_dst, a_dst,
                                                     ps_av[0:HD, :])
                                nc.vector.tensor_add(d_dst, d_dst,
                                                     ps_av[HD:HD + 1, :])

            # ---- Phase C: softmax normalization + output projection ----
            with (
                tc.tile_pool(name="pc_sb", bufs=3) as pc_sb,
                tc.tile_pool(name="pc_w", bufs=1) as pc_w,
                tc.tile_pool(name="pc_ps", bufs=2, space="PSUM") as pc_ps,
            ):
                for h in range(H):
                    dp = 32 * (h % 4)
                    d_row = den[dp:dp + 1, h // 4, :]
                    nc.vector.reciprocal(d_row, d_row)
                for hp in range(NHP):
                    rcp_b = pc_sb.tile([P, TQ], f32, tag="rcp_b", bufs=2)
                    for i in range(2):
                        h = 2 * hp + i
                        dp = 32 * (h % 4)
                        nc.gpsimd.partition_broadcast(
                            rcp_b[i * HD:(i + 1) * HD, :],
                            den[dp:dp + 1, h // 4, :])
                    nc.vector.tensor_mul(attnT[:, hp, :], attnT[:, hp, :],
                                         rcp_b[:])

                wp_t = pc_w.tile([P, NCT, C], f32)
                for hp in range(NCT):
                    nc.sync.dma_start(wp_t[:, hp, :],
                                      wpT[hp * P:(hp + 1) * P, :])
                for ct in range(NCT):
                    ps = pc_ps.tile([P, TQ], f32, tag="proj_ps")
                    for hp in range(NHP):
                        nc.tensor.matmul(
                            ps[:], _r(wp_t[:, hp, ct * P:(ct + 1) * P]),
                            _r(attnT[:, hp, :]),
                            start=(hp == 0), stop=(hp == NHP - 1))
                    o = x2T[:, ct, :]
                    nc.vector.tensor_scalar_add(o, ps[:], bp_t[:, ct:ct + 1])
                    nc.vector.tensor_add(o, o, xq_t[:, ct, :])

        # ---- Phase D: LN2, fc1+gelu, fc2 + residual ----
        with (
            tc.tile_pool(name="pd_sb", bufs=3) as pd_sb,
            tc.tile_pool(name="pd_ln", bufs=1) as pd_ln,
            tc.tile_pool(name="pd_g", bufs=1) as pd_g,
            tc.tile_pool(name="pd_w", bufs=2) as pd_w,
            tc.tile_pool(name="pd_ps", bufs=2, space="PSUM") as pd_ps,
            tc.tile_pool(name="pd_ps2", bufs=1, space="PSUM") as pd_ps2,
        ):
            ln2T = pd_ln.tile([P, NCT, TQ], f32)
            _emit_ln(nc, ones_stat, eps_t, pd_ps, pd_sb,
                     lambda ct: x2T[:, ct, :], ln2T, g2_t, bb2_t, TQ)

            g1T = pd_g.tile([P, NHT, TQ], f32)
            for htg in range(NHT // 4):
                w1_t = pd_w.tile([P, NCT, 512], f32, tag="w1")
                for ct in range(NCT):
                    nc.sync.dma_start(
                        w1_t[:, ct, :],
                        w1T[ct * P:(ct + 1) * P, htg * 512:(htg + 1) * 512])
                for hl in range(4):
                    ht = htg * 4 + hl
                    ps = pd_ps.tile([P, TQ], f32, tag="fc1_ps")
                    for ct in range(NCT):
                        nc.tensor.matmul(
                            ps[:], _r(w1_t[:, ct, hl * P:(hl + 1) * P]),
                            _r(ln2T[:, ct, :]),
                            start=(ct == 0), stop=(ct == NCT - 1))
                    nc.scalar.activation(g1T[:, ht, :], ps[:], gelu_fn,
                                         bias=b1_t[:, ht:ht + 1])

            for ctg in range(2):
                ps_out = [pd_ps2.tile([P, TQ], f32, tag=f"fc2_{i}",
                                      name=f"fc2_ps_{i}")
                          for i in range(4)]
                for ht in range(NHT):
                    w2_t = pd_w.tile([P, 512], f32, tag="w2", bufs=3)
                    nc.sync.dma_start(
                        w2_t[:],
                        w2T[ht * P:(ht + 1) * P, ctg * 512:(ctg + 1) * 512])
                    for cl in range(4):
                        nc.tensor.matmul(
                            ps_out[cl][:], _r(w2_t[:, cl * P:(cl + 1) * P]),
                            _r(g1T[:, ht, :]),
                            start=(ht == 0), stop=(ht == NHT - 1))
                for cl in range(4):
                    ct = ctg * 4 + cl
                    o = pd_sb.tile([P, TQ], f32, tag="out_t")
                    nc.vector.tensor_scalar_add(o[:], ps_out[cl][:],
                                                b2_t[:, ct:ct + 1])
                    nc.vector.tensor_add(o[:], o[:], x2T[:, ct, :])
                    nc.sync.dma_start(outT[ct * P:(ct + 1) * P, :], o[:])

    nc.finalize()
    return nc


_program = None


def _get_program():
    global _program
    if _program is None:
        _program = build_program()
    return _program


def kernel(**inputs):
    x = np.asarray(inputs["x"], dtype=np.float32)
    B, N, _ = x.shape  # [2, 2048, 1024]

    def T(a):
        return np.ascontiguousarray(np.asarray(a, dtype=np.float32).T)

    w_qkv = np.asarray(inputs["w_qkv"], dtype=np.float32)
    shared = {
        "wqT": T(w_qkv[0:C]),
        "wkT": T(w_qkv[C:2 * C]),
        "wvT": T(w_qkv[2 * C:3 * C]),
        "wpT": T(inputs["w_proj"]),
        "w1T": T(inputs["w_fc1"]),
        "w2T": T(inputs["w_fc2"]),
        "bp": np.asarray(inputs["b_proj"], dtype=np.float32),
        "b1": np.asarray(inputs["b_fc1"], dtype=np.float32),
        "b2": np.asarray(inputs["b_fc2"], dtype=np.float32),
        "ln1g": np.asarray(inputs["ln1_g"], dtype=np.float32),
        "ln1b": np.asarray(inputs["ln1_b"], dtype=np.float32),
        "ln2g": np.asarray(inputs["ln2_g"], dtype=np.float32),
        "ln2b": np.asarray(inputs["ln2_b"], dtype=np.float32),
    }
    xT = [T(x[b]) for b in range(B)]  # [C, N] each
    in_maps = []
    for core in range(8):
        b, qb = core // 4, core % 4
        m = dict(shared)
        m["xkvT"] = xT[b]
        m["xqT"] = np.ascontiguousarray(xT[b][:, qb * TQ:(qb + 1) * TQ])
        in_maps.append(m)

    nc = _get_program()
    res = run_bass_kernel_spmd(nc, in_maps, list(range(8)))

    out = np.empty((B, N, C), dtype=np.float32)
    for core in range(8):
        b, qb = core // 4, core % 4
        out[b, qb * TQ:(qb + 1) * TQ, :] = res.results[core]["outT"].T
    return out
